# revision 14
# baseline (speedup 1.0000x reference)
"""Causal multi-head attention (B=2, T=2048, D=1024, NH=16, HD=64) on 8 trn2
NeuronCores.

Sharding: data-parallel over batch (2) x tensor-parallel over head groups (4),
Megatron-style. Core c handles batch c//4, heads 4*(c%4)..4*(c%4)+3. The host
sums the 4 partial projections per batch.

Layout is feature-on-partition throughout (x^T, qk^T, S^T [k,q], O^T, out^T).
All matmul inputs are bf16 (halves HBM traffic, enables fast weight load);
PSUM accumulation is f32.

Single software pipeline per head-pair:
  S^T matmuls (K=64, two heads row-tiled into the PE concurrently) -> exp on
  the scalar engine (scale=1/8 fused; softmax max-subtraction skipped, scores
  are O(1)) -> causal zeroing of the 128-wide diagonal window only (gpsimd
  affine_select) -> PV (two heads col-tiled, M=64 each) and Z accumulation
  (ones lhsT broadcasts Z across each head's 64 output rows, col-tiled) ->
  normalize = one DVE reciprocal + one DVE multiply per (pair, qc).
Pair 0's pipeline is fed early (only its q/k m-tiles precede it); the V
projection, pair-1 qkv m-tiles, and output projection pieces are interleaved
into the attention steps as PE filler so the tensor engine never idles long
enough for the HAM clock gate to re-throttle.
"""

import sys

if "/opt/trn_rl_repo" not in sys.path:
    sys.path.insert(0, "/opt/trn_rl_repo")

import numpy as np
import ml_dtypes
import concourse.mybir as mybir
from concourse import bacc
from concourse.tile import TileContext
from concourse import bass_utils

B, T, D = 2, 2048, 1024
NH, HD = 16, 64
N_CORES = 8

KT = D // 128  # 8 contraction tiles over model dim
TT = T // 128  # 16 t-blocks of 128

BF16 = mybir.dt.bfloat16
F32 = mybir.dt.float32
NPBF = ml_dtypes.bfloat16

DEPTH = 5  # S->PV pipeline lag in steps


def build_nc():
    nc = bacc.Bacc()
    xT = nc.dram_tensor("xT", [D, T], BF16, kind="ExternalInput")
    wqk = nc.dram_tensor("wqk", [D, 512], BF16, kind="ExternalInput")
    wv = nc.dram_tensor("wv", [D, 256], BF16, kind="ExternalInput")
    wp = nc.dram_tensor("wp", [256, D], BF16, kind="ExternalInput")
    outT = nc.dram_tensor("outT", [4, D, 512], BF16, kind="ExternalOutput")

    with TileContext(nc) as tc:
        with (
            tc.tile_pool(name="persist", bufs=1) as pers,
            tc.tile_pool(name="stage", bufs=1) as stg,
            tc.tile_pool(name="miscp", bufs=2, space="PSUM") as misc,
        ):
            qkT = [
                pers.tile([128, T], BF16, tag=f"qkT{mt}", name=f"qkT{mt}")
                for mt in range(4)
            ]
            V_sb = [
                pers.tile([128, 256], BF16, tag=f"V{tt}", name=f"V{tt}")
                for tt in range(TT)
            ]
            AT = [
                pers.tile([128, T], BF16, tag=f"AT{p}", name=f"AT{p}")
                for p in range(2)
            ]
            wp_sb = [
                pers.tile([128, D], BF16, tag=f"wp{p}", name=f"wp{p}")
                for p in range(2)
            ]
            ones64 = pers.tile([128, 64], BF16, tag="ones", name="ones64")
            nc.vector.memset(ones64, 1.0)

            with tc.tile_pool(name="qkv_in", bufs=1) as qin:
                wqk_sb, wv_sb, xT_sb = [], [], []
                dmaq = [nc.sync, nc.scalar, nc.gpsimd]
                # weights first (small; vg/proj depend on them), then x^T in
                # column-quarters so the first qkv matmuls start after ~1/4
                # of the stream
                for kt in range(KT):
                    twqk = qin.tile([128, 512], BF16, tag=f"wqk{kt}", name=f"wqk{kt}")
                    dmaq[kt % 3].dma_start(
                        out=twqk, in_=wqk[kt * 128 : (kt + 1) * 128, :]
                    )
                    wqk_sb.append(twqk)
                    txT = qin.tile([128, T], BF16, tag=f"xT{kt}", name=f"xT{kt}")
                    xT_sb.append(txT)
                    twv = qin.tile([128, 256], BF16, tag=f"wv{kt}", name=f"wv{kt}")
                    dmaq[(kt + 1) % 3].dma_start(
                        out=twv, in_=wv[kt * 128 : (kt + 1) * 128, :]
                    )
                    wv_sb.append(twv)
                for p in range(2):
                    dmaq[p].dma_start(
                        out=wp_sb[p], in_=wp[p * 128 : (p + 1) * 128, :]
                    )
                for kt in range(KT):
                    dmaq[kt % 3].dma_start(
                        out=xT_sb[kt], in_=xT[kt * 128 : (kt + 1) * 128, :]
                    )

                # ---- building blocks -----------------------------------
                copy_flip = [0]

                def emit_qkq(mt, q, phase1):
                    """One [128,512] quarter of qk^T m-tile mt."""
                    psq = misc.tile([128, 512], F32, tag="mp", name=f"q{mt}_{q}")
                    for kt in range(KT):
                        nc.tensor.matmul(
                            psq,
                            wqk_sb[kt][:, mt * 128 : (mt + 1) * 128],
                            xT_sb[kt][:, q * 512 : (q + 1) * 512],
                            start=(kt == 0),
                            stop=(kt == KT - 1),
                        )
                    dst = qkT[mt][:, q * 512 : (q + 1) * 512]
                    if phase1 and copy_flip[0] % 2 == 0:
                        nc.scalar.copy(dst, psq)
                    else:
                        nc.vector.tensor_copy(dst, psq)
                    copy_flip[0] += 1

                def emit_vg(g, phase1):
                    """V for t-blocks 2g, 2g+1 (all 4 heads)."""
                    psv = misc.tile([128, 512], F32, tag="mp", name=f"v{g}")
                    for j in range(2):
                        tt = 2 * g + j
                        for kt in range(KT):
                            nc.tensor.matmul(
                                psv[:, j * 256 : (j + 1) * 256],
                                xT_sb[kt][:, tt * 128 : (tt + 1) * 128],
                                wv_sb[kt],
                                start=(kt == 0),
                                stop=(kt == KT - 1),
                            )
                    for j in range(2):
                        dst = V_sb[2 * g + j]
                        if phase1 and copy_flip[0] % 2 == 0:
                            nc.scalar.copy(dst, psv[:, j * 256 : (j + 1) * 256])
                        else:
                            nc.vector.tensor_copy(
                                dst, psv[:, j * 256 : (j + 1) * 256]
                            )
                        copy_flip[0] += 1

                # Micro-sliced fillers: ~0.45us of PE work per unit so a unit
                # hides in one attention step's tensor-engine slack instead of
                # stretching the ACT-paced pipeline.
                def qkq_units(mt, q):
                    state = {}

                    def mk(u):
                        def unit():
                            if u == 0:
                                state["ps"] = misc.tile(
                                    [128, 512], F32, tag="mp", name=f"uq{mt}_{q}"
                                )
                            ps = state["ps"]
                            for kt in (2 * u, 2 * u + 1):
                                nc.tensor.matmul(
                                    ps,
                                    wqk_sb[kt][:, mt * 128 : (mt + 1) * 128],
                                    xT_sb[kt][:, q * 512 : (q + 1) * 512],
                                    start=(kt == 0),
                                    stop=(kt == KT - 1),
                                )
                            if u == 3:
                                nc.vector.tensor_copy(
                                    qkT[mt][:, q * 512 : (q + 1) * 512], ps
                                )

                        return unit

                    return [mk(u) for u in range(4)]

                def vg_units(g):
                    state = {}

                    def mk(u):
                        j, half = divmod(u, 2)

                        def unit():
                            if u == 0:
                                state["ps"] = misc.tile(
                                    [128, 512], F32, tag="mp", name=f"uv{g}"
                                )
                            ps = state["ps"]
                            tt = 2 * g + j
                            for kt in range(4 * half, 4 * half + 4):
                                nc.tensor.matmul(
                                    ps[:, j * 256 : (j + 1) * 256],
                                    xT_sb[kt][:, tt * 128 : (tt + 1) * 128],
                                    wv_sb[kt],
                                    start=(kt == 0),
                                    stop=(kt == KT - 1),
                                )
                            if half == 1:
                                nc.vector.tensor_copy(
                                    V_sb[tt], ps[:, j * 256 : (j + 1) * 256]
                                )

                        return unit

                    return [mk(u) for u in range(4)]

                def emit_proj(qc, jt2, sub, late=False):
                    c0 = (2 * jt2 + sub) * 128
                    psp = misc.tile([128, 512], F32, tag="mp", name=f"pp{qc}{jt2}{sub}")
                    for p in range(2):
                        nc.tensor.matmul(
                            psp,
                            wp_sb[p][:, c0 : c0 + 128],
                            AT[p][:, qc * 512 : (qc + 1) * 512],
                            start=(p == 0),
                            stop=(p == 1),
                        )
                    ost = stg.tile(
                        [128, 512], BF16, tag="ost", bufs=4, name=f"ost{qc}{jt2}{sub}"
                    )
                    # the tail pieces run after the last exp: use the idle
                    # scalar engine for half the psum->sbuf casts so pieces
                    # pipeline instead of serializing on the vector engine
                    if late and copy_flip[0] % 2 == 0:
                        nc.scalar.copy(ost, psp)
                    else:
                        nc.vector.tensor_copy(ost, psp)
                    copy_flip[0] += 1
                    nc.sync.dma_start(
                        out=outT[qc, c0 : c0 + 128, :], in_=ost
                    )

                # ---- phase 1: just enough for pair-0 qc0 to start ------
                # (first q and k column-quarters, kt-interleaved so each x^T
                # tile is consumed as it lands); all remaining qkv work
                # streams in as deadline-ordered fillers
                ps00 = misc.tile([128, 512], F32, tag="mp", name="ps00")
                ps20 = misc.tile([128, 512], F32, tag="mp", name="ps20")
                for kt in range(KT):
                    for mt, ps in ((0, ps00), (2, ps20)):
                        nc.tensor.matmul(
                            ps,
                            wqk_sb[kt][:, mt * 128 : (mt + 1) * 128],
                            xT_sb[kt][:, 0:512],
                            start=(kt == 0),
                            stop=(kt == KT - 1),
                        )
                nc.scalar.copy(qkT[0][:, 0:512], ps00)
                nc.vector.tensor_copy(qkT[2][:, 0:512], ps20)

                # ---- phases 2+3: per-pair attention pipelines ----------
                with (
                    tc.tile_pool(name="ptile", bufs=DEPTH + 2) as ppool,
                    tc.tile_pool(name="psS", bufs=2, space="PSUM") as pss,
                    tc.tile_pool(name="psO", bufs=1, space="PSUM") as pso,
                    tc.tile_pool(name="psZ", bufs=1, space="PSUM") as psz,
                ):
                    # ascending qc order: each qc's inputs need only one more
                    # x^T column-quarter than the previous, so the pipeline
                    # starts as soon as quarter 0 lands
                    QC_ORDER = [0, 1, 2, 3]
                    steps = [
                        (qc, kb) for qc in QC_ORDER for kb in range(4 * qc + 4)
                    ]

                    def pair_phase(p, fillers):
                        pts = {}
                        cur = {}
                        proj_q = []
                        filler_q = list(fillers)

                        def emit_S(qc, kb):
                            off = 128 * (kb - 4 * qc)
                            lo = max(off, 0)
                            psS = pss.tile(
                                [128, 2, 512], F32, tag="s", name=f"s{p}{qc}{kb}"
                            )
                            qT, kT = qkT[p], qkT[2 + p]
                            for h in range(2):
                                nc.tensor.matmul(
                                    psS[:, h, lo:512],
                                    kT[
                                        64 * h : 64 * h + 64,
                                        kb * 128 : (kb + 1) * 128,
                                    ],
                                    qT[
                                        64 * h : 64 * h + 64,
                                        qc * 512 + lo : (qc + 1) * 512,
                                    ],
                                    start=True,
                                    stop=True,
                                )
                            pt = ppool.tile(
                                [128, 2, 512], BF16, tag="pt", name=f"pt{p}{qc}{kb}"
                            )
                            nc.scalar.activation(
                                pt[:, :, lo:512],
                                psS[:, :, lo:512],
                                mybir.ActivationFunctionType.Exp,
                                scale=0.125,
                            )
                            if off >= 0:
                                for h in range(2):
                                    nc.gpsimd.affine_select(
                                        pt[:, h, lo : lo + 128],
                                        pt[:, h, lo : lo + 128],
                                        pattern=[[1, 128]],
                                        compare_op=mybir.AluOpType.is_ge,
                                        fill=0.0,
                                        base=0,
                                        channel_multiplier=-1,
                                    )
                            pts[(qc, kb)] = pt

                        def emit_PV(qc, kb):
                            off = 128 * (kb - 4 * qc)
                            lo = max(off, 0)
                            if kb == 0:
                                cur["o"] = pso.tile(
                                    [128, 512], F32, tag="o", name=f"o{p}{qc}"
                                )
                                cur["z"] = psz.tile(
                                    [128, 512], F32, tag="z", name=f"z{p}{qc}"
                                )
                            oacc, zacc = cur["o"], cur["z"]
                            pt = pts.pop((qc, kb))
                            last = kb == 4 * qc + 3
                            for h in range(2):
                                nc.tensor.matmul(
                                    oacc[64 * h : 64 * h + 64, lo:512],
                                    V_sb[kb][:, (2 * p + h) * 64 : (2 * p + h + 1) * 64],
                                    pt[:, h, lo:512],
                                    start=(kb == 0),
                                    stop=last,
                                )
                            for h in range(2):
                                nc.tensor.matmul(
                                    zacc[64 * h : 64 * h + 64, lo:512],
                                    ones64,
                                    pt[:, h, lo:512],
                                    start=(kb == 0),
                                    stop=last,
                                )
                            if last:
                                zrec = stg.tile(
                                    [128, 512], F32, tag="zr", bufs=2,
                                    name=f"zr{p}{qc}",
                                )
                                nc.vector.reciprocal_approx_fast(zrec, zacc)
                                nc.vector.tensor_mul(
                                    AT[p][:, qc * 512 : (qc + 1) * 512],
                                    oacc,
                                    zrec,
                                )
                                if p == 1:
                                    for jt2 in range(4):
                                        for sub in range(2):
                                            proj_q.append((qc, jt2, sub))

                        for i in range(len(steps) + DEPTH):
                            if i < len(steps):
                                qc, kb = steps[i]
                                emit_S(qc, kb)
                                # ~0.45us filler units hide in each step's
                                # tensor slack; short (diagonal) steps have
                                # room for two
                                budget = 3 if qc == 0 else 2
                                for _ in range(budget):
                                    if filler_q:
                                        filler_q.pop(0)()
                                    elif proj_q:
                                        emit_proj(*proj_q.pop(0))
                            else:
                                # flush region: drain remaining work
                                for _ in range(2):
                                    if filler_q:
                                        filler_q.pop(0)()
                                    elif proj_q:
                                        emit_proj(*proj_q.pop(0), late=True)
                            j = i - DEPTH
                            if j >= 0:
                                emit_PV(*steps[j])
                        while filler_q:
                            filler_q.pop(0)()
                        while proj_q:
                            emit_proj(*proj_q.pop(0), late=True)

                    # filler unit lists, ordered by data deadline against the
                    # ascending-qc step schedule (qT quarter n by qc_n start;
                    # kT quarter n by qc_n's kb=4n step; V[tt] by its first PV)
                    fillers0 = []
                    fillers0 += vg_units(0) + vg_units(1)
                    fillers0 += qkq_units(0, 1) + qkq_units(2, 1)
                    fillers0 += vg_units(2) + qkq_units(0, 2)
                    fillers0 += vg_units(3) + qkq_units(2, 2)
                    fillers0 += qkq_units(0, 3)
                    fillers0 += vg_units(4) + vg_units(5)
                    fillers0 += qkq_units(2, 3)
                    fillers0 += vg_units(6) + vg_units(7)
                    fillers0 += qkq_units(3, 0) + qkq_units(1, 0)
                    fillers1 = []
                    for mt, q in [(1, 1), (3, 1), (1, 2), (3, 2), (1, 3), (3, 3)]:
                        fillers1 += qkq_units(mt, q)
                    pair_phase(0, fillers0)
                    pair_phase(1, fillers1)

    nc.finalize()
    return nc


_NC_CACHE = None


def _get_nc():
    global _NC_CACHE
    if _NC_CACHE is None:
        _NC_CACHE = build_nc()
    return _NC_CACHE


def make_in_maps(x, w_qkv, w_proj):
    x = np.asarray(x, dtype=np.float32)
    w_qkv = np.asarray(w_qkv, dtype=np.float32)
    w_proj = np.asarray(w_proj, dtype=np.float32)
    in_maps = []
    for c in range(N_CORES):
        b, g = divmod(c, 4)
        cs = 256 * g
        in_maps.append(
            {
                "xT": np.ascontiguousarray(x[b].T).astype(NPBF),
                "wqk": np.ascontiguousarray(
                    np.concatenate(
                        [w_qkv[:, cs : cs + 256], w_qkv[:, D + cs : D + cs + 256]],
                        axis=1,
                    )
                ).astype(NPBF),
                "wv": np.ascontiguousarray(
                    w_qkv[:, 2 * D + cs : 2 * D + cs + 256]
                ).astype(NPBF),
                "wp": np.ascontiguousarray(w_proj[cs : cs + 256, :]).astype(NPBF),
            }
        )
    return in_maps


def assemble(results):
    out = np.empty((B, T, D), dtype=np.float32)
    for b in range(B):
        acc = results[4 * b]["outT"].astype(np.float32)
        for g in range(1, 4):
            acc = acc + results[4 * b + g]["outT"].astype(np.float32)
        for qc in range(4):
            out[b, qc * 512 : (qc + 1) * 512, :] = acc[qc].T
    return out


def kernel(x, w_qkv, w_proj, trace=False):
    nc = _get_nc()
    in_maps = make_in_maps(x, w_qkv, w_proj)
    res = bass_utils.run_bass_kernel_spmd(
        nc, in_maps, core_ids=list(range(N_CORES)), trace=trace
    )
    out = assemble(res.results)
    if trace:
        kernel.last_exec_time_ns = res.exec_time_ns
        kernel.last_result = res
    return out


# revision 18
# speedup vs baseline: 1.1788x; 1.1788x over previous
"""Causal multi-head attention (B=2, T=2048, D=1024, NH=16, HD=64) on 8 trn2
NeuronCores.

Sharding: data-parallel over batch (2) x tensor-parallel over head groups (4),
Megatron-style. Core c handles batch c//4, heads 4*(c%4)..4*(c%4)+3. The host
sums the 4 partial projections per batch.

Layout is feature-on-partition throughout (x^T, qk^T, S^T [k,q], O^T, out^T).
All matmul inputs are bf16 (halves HBM traffic, enables fast weight load);
PSUM accumulation is f32.

Single software pipeline per head-pair:
  S^T matmuls (K=64, two heads row-tiled into the PE concurrently) -> exp on
  the scalar engine (scale=1/8 fused; softmax max-subtraction skipped, scores
  are O(1)) -> causal zeroing of the 128-wide diagonal window only (gpsimd
  affine_select) -> PV (two heads col-tiled, M=64 each) and Z accumulation
  (ones lhsT broadcasts Z across each head's 64 output rows, col-tiled) ->
  normalize = one DVE reciprocal + one DVE multiply per (pair, qc).
Pair 0's pipeline is fed early (only its q/k m-tiles precede it); the V
projection, pair-1 qkv m-tiles, and output projection pieces are interleaved
into the attention steps as PE filler so the tensor engine never idles long
enough for the HAM clock gate to re-throttle.
"""

import sys

if "/opt/trn_rl_repo" not in sys.path:
    sys.path.insert(0, "/opt/trn_rl_repo")

import numpy as np
import ml_dtypes
import concourse.mybir as mybir
from concourse import bacc
from concourse.tile import TileContext
from concourse import bass_utils

B, T, D = 2, 2048, 1024
NH, HD = 16, 64
N_CORES = 8

KT = D // 128  # 8 contraction tiles over model dim
TT = T // 128  # 16 t-blocks of 128

BF16 = mybir.dt.bfloat16
F32 = mybir.dt.float32
NPBF = ml_dtypes.bfloat16

DEPTH = 5  # S->PV pipeline lag in steps


def build_nc():
    nc = bacc.Bacc()
    xT = nc.dram_tensor("xT", [D, T], BF16, kind="ExternalInput")
    wqk = nc.dram_tensor("wqk", [D, 512], BF16, kind="ExternalInput")
    wv = nc.dram_tensor("wv", [D, 256], BF16, kind="ExternalInput")
    wp = nc.dram_tensor("wp", [256, D], BF16, kind="ExternalInput")
    outT = nc.dram_tensor("outT", [4, D, 512], BF16, kind="ExternalOutput")

    with TileContext(nc) as tc:
        with (
            tc.tile_pool(name="persist", bufs=1) as pers,
            tc.tile_pool(name="stage", bufs=1) as stg,
            tc.tile_pool(name="miscp", bufs=2, space="PSUM") as misc,
        ):
            qkT = [
                pers.tile([128, T], BF16, tag=f"qkT{mt}", name=f"qkT{mt}")
                for mt in range(4)
            ]
            V_sb = [
                pers.tile([128, 256], BF16, tag=f"V{tt}", name=f"V{tt}")
                for tt in range(TT)
            ]
            AT = [
                pers.tile([128, T], BF16, tag=f"AT{p}", name=f"AT{p}")
                for p in range(2)
            ]
            wp_sb = [
                pers.tile([128, D], BF16, tag=f"wp{p}", name=f"wp{p}")
                for p in range(2)
            ]
            ones64 = pers.tile([128, 64], BF16, tag="ones", name="ones64")
            nc.vector.memset(ones64, 1.0)

            with tc.tile_pool(name="qkv_in", bufs=1) as qin:
                wqk_sb, wv_sb, xT_sb = [], [], []
                dmaq = [nc.sync, nc.scalar, nc.gpsimd]
                # weights first (small; vg/proj depend on them), then x^T in
                # column-quarters so the first qkv matmuls start after ~1/4
                # of the stream
                for kt in range(KT):
                    twqk = qin.tile([128, 512], BF16, tag=f"wqk{kt}", name=f"wqk{kt}")
                    dmaq[kt % 3].dma_start(
                        out=twqk, in_=wqk[kt * 128 : (kt + 1) * 128, :]
                    )
                    wqk_sb.append(twqk)
                    txT = qin.tile([128, T], BF16, tag=f"xT{kt}", name=f"xT{kt}")
                    xT_sb.append(txT)
                    twv = qin.tile([128, 256], BF16, tag=f"wv{kt}", name=f"wv{kt}")
                    wv_sb.append(twv)
                for half in range(2):
                    for kt in range(KT):
                        dmaq[kt % 3].dma_start(
                            out=xT_sb[kt][:, half * 1024 : (half + 1) * 1024],
                            in_=xT[
                                kt * 128 : (kt + 1) * 128,
                                half * 1024 : (half + 1) * 1024,
                            ],
                        )
                    if half == 0:
                        # wv/wp after the x halves phase 1 needs
                        for kt in range(KT):
                            dmaq[(kt + 1) % 3].dma_start(
                                out=wv_sb[kt],
                                in_=wv[kt * 128 : (kt + 1) * 128, :],
                            )
                        for p in range(2):
                            dmaq[p].dma_start(
                                out=wp_sb[p], in_=wp[p * 128 : (p + 1) * 128, :]
                            )

                # ---- building blocks -----------------------------------
                copy_flip = [0]

                def emit_qkq(mt, q, phase1):
                    """One [128,512] quarter of qk^T m-tile mt."""
                    psq = misc.tile([128, 512], F32, tag="mp", name=f"q{mt}_{q}")
                    for kt in range(KT):
                        nc.tensor.matmul(
                            psq,
                            wqk_sb[kt][:, mt * 128 : (mt + 1) * 128],
                            xT_sb[kt][:, q * 512 : (q + 1) * 512],
                            start=(kt == 0),
                            stop=(kt == KT - 1),
                        )
                    dst = qkT[mt][:, q * 512 : (q + 1) * 512]
                    if phase1 and copy_flip[0] % 2 == 0:
                        nc.scalar.copy(dst, psq)
                    else:
                        nc.vector.tensor_copy(dst, psq)
                    copy_flip[0] += 1

                def emit_vg(g, phase1):
                    """V for t-blocks 2g, 2g+1 (all 4 heads)."""
                    psv = misc.tile([128, 512], F32, tag="mp", name=f"v{g}")
                    for j in range(2):
                        tt = 2 * g + j
                        for kt in range(KT):
                            nc.tensor.matmul(
                                psv[:, j * 256 : (j + 1) * 256],
                                xT_sb[kt][:, tt * 128 : (tt + 1) * 128],
                                wv_sb[kt],
                                start=(kt == 0),
                                stop=(kt == KT - 1),
                            )
                    for j in range(2):
                        dst = V_sb[2 * g + j]
                        if phase1 and copy_flip[0] % 2 == 0:
                            nc.scalar.copy(dst, psv[:, j * 256 : (j + 1) * 256])
                        else:
                            nc.vector.tensor_copy(
                                dst, psv[:, j * 256 : (j + 1) * 256]
                            )
                        copy_flip[0] += 1

                # Micro-sliced fillers: ~0.45us of PE work per unit so a unit
                # hides in one attention step's tensor-engine slack instead of
                # stretching the ACT-paced pipeline.
                def qkq_units(mt, q):
                    state = {}

                    def mk(u):
                        def unit():
                            if u == 0:
                                state["ps"] = misc.tile(
                                    [128, 512], F32, tag="mp", name=f"uq{mt}_{q}"
                                )
                            ps = state["ps"]
                            for kt in (2 * u, 2 * u + 1):
                                nc.tensor.matmul(
                                    ps,
                                    wqk_sb[kt][:, mt * 128 : (mt + 1) * 128],
                                    xT_sb[kt][:, q * 512 : (q + 1) * 512],
                                    start=(kt == 0),
                                    stop=(kt == KT - 1),
                                )
                            if u == 3:
                                nc.vector.tensor_copy(
                                    qkT[mt][:, q * 512 : (q + 1) * 512], ps
                                )

                        unit.produces = ("qk", mt, q) if u == 3 else None
                        return unit

                    return [mk(u) for u in range(4)]

                def vg_units(g):
                    state = {}

                    def mk(u):
                        j, half = divmod(u, 2)

                        def unit():
                            if u == 0:
                                state["ps"] = misc.tile(
                                    [128, 512], F32, tag="mp", name=f"uv{g}"
                                )
                            ps = state["ps"]
                            tt = 2 * g + j
                            for kt in range(4 * half, 4 * half + 4):
                                nc.tensor.matmul(
                                    ps[:, j * 256 : (j + 1) * 256],
                                    xT_sb[kt][:, tt * 128 : (tt + 1) * 128],
                                    wv_sb[kt],
                                    start=(kt == 0),
                                    stop=(kt == KT - 1),
                                )
                            if half == 1:
                                nc.vector.tensor_copy(
                                    V_sb[tt], ps[:, j * 256 : (j + 1) * 256]
                                )

                        unit.produces = ("v", 2 * g + j) if half == 1 else None
                        return unit

                    return [mk(u) for u in range(4)]

                def emit_proj(qc, jt2, sub, late=False):
                    c0 = (2 * jt2 + sub) * 128
                    psp = misc.tile([128, 512], F32, tag="mp", name=f"pp{qc}{jt2}{sub}")
                    for p in range(2):
                        nc.tensor.matmul(
                            psp,
                            wp_sb[p][:, c0 : c0 + 128],
                            AT[p][:, qc * 512 : (qc + 1) * 512],
                            start=(p == 0),
                            stop=(p == 1),
                        )
                    ost = stg.tile(
                        [128, 512], BF16, tag="ost", bufs=4, name=f"ost{qc}{jt2}{sub}"
                    )
                    # the tail pieces run after the last exp: use the idle
                    # scalar engine for half the psum->sbuf casts so pieces
                    # pipeline instead of serializing on the vector engine
                    if late and copy_flip[0] % 2 == 0:
                        nc.scalar.copy(ost, psp)
                    else:
                        nc.vector.tensor_copy(ost, psp)
                    copy_flip[0] += 1
                    ([nc.sync, nc.gpsimd][copy_flip[0] % 2]).dma_start(
                        out=outT[qc, c0 : c0 + 128, :], in_=ost
                    )

                # ---- phase 1: just enough for pair-0 qc0 to start ------
                # (first q and k column-quarters, kt-interleaved so each x^T
                # tile is consumed as it lands); all remaining qkv work
                # streams in as deadline-ordered fillers
                ps00 = misc.tile([128, 512], F32, tag="mp", name="ps00")
                ps20 = misc.tile([128, 512], F32, tag="mp", name="ps20")
                for kt in range(KT):
                    for mt, ps in ((0, ps00), (2, ps20)):
                        nc.tensor.matmul(
                            ps,
                            wqk_sb[kt][:, mt * 128 : (mt + 1) * 128],
                            xT_sb[kt][:, 0:512],
                            start=(kt == 0),
                            stop=(kt == KT - 1),
                        )
                nc.scalar.copy(qkT[0][:, 0:512], ps00)
                nc.vector.tensor_copy(qkT[2][:, 0:512], ps20)
                emit_qkq(0, 1, True)

                # ---- phases 2+3: per-pair attention pipelines ----------
                with (
                    tc.tile_pool(name="ptile", bufs=DEPTH + 2) as ppool,
                    tc.tile_pool(name="psS", bufs=2, space="PSUM") as pss,
                    tc.tile_pool(name="psO", bufs=1, space="PSUM") as pso,
                    tc.tile_pool(name="psZ", bufs=1, space="PSUM") as psz,
                ):
                    # ascending qc order: each qc's inputs need only one more
                    # x^T column-quarter than the previous, so the pipeline
                    # starts as soon as quarter 0 lands
                    QC_ORDER = [0, 1, 2, 3]
                    steps = [
                        (qc, kb) for qc in QC_ORDER for kb in range(4 * qc + 4)
                    ]

                    made = {("qk", 0, 0), ("qk", 2, 0), ("qk", 0, 1)}

                    def pair_phase(p, fillers):
                        pts = {}
                        cur = {}
                        proj_q = []
                        filler_q = list(fillers)

                        def emit_S(qc, kb):
                            off = 128 * (kb - 4 * qc)
                            lo = max(off, 0)
                            psS = pss.tile(
                                [128, 2, 512], F32, tag="s", name=f"s{p}{qc}{kb}"
                            )
                            qT, kT = qkT[p], qkT[2 + p]
                            for h in range(2):
                                nc.tensor.matmul(
                                    psS[:, h, lo:512],
                                    kT[
                                        64 * h : 64 * h + 64,
                                        kb * 128 : (kb + 1) * 128,
                                    ],
                                    qT[
                                        64 * h : 64 * h + 64,
                                        qc * 512 + lo : (qc + 1) * 512,
                                    ],
                                    start=True,
                                    stop=True,
                                )
                            pt = ppool.tile(
                                [128, 2, 512], BF16, tag="pt", name=f"pt{p}{qc}{kb}"
                            )
                            nc.scalar.activation(
                                pt[:, :, lo:512],
                                psS[:, :, lo:512],
                                mybir.ActivationFunctionType.Exp,
                                scale=0.125,
                            )
                            if off >= 0:
                                for h in range(2):
                                    nc.gpsimd.affine_select(
                                        pt[:, h, lo : lo + 128],
                                        pt[:, h, lo : lo + 128],
                                        pattern=[[1, 128]],
                                        compare_op=mybir.AluOpType.is_ge,
                                        fill=0.0,
                                        base=0,
                                        channel_multiplier=-1,
                                    )
                            pts[(qc, kb)] = pt

                        def emit_PV(qc, kb):
                            off = 128 * (kb - 4 * qc)
                            lo = max(off, 0)
                            if kb == 0:
                                cur["o"] = pso.tile(
                                    [128, 512], F32, tag="o", name=f"o{p}{qc}"
                                )
                                cur["z"] = psz.tile(
                                    [128, 512], F32, tag="z", name=f"z{p}{qc}"
                                )
                            oacc, zacc = cur["o"], cur["z"]
                            pt = pts.pop((qc, kb))
                            last = kb == 4 * qc + 3
                            for h in range(2):
                                nc.tensor.matmul(
                                    oacc[64 * h : 64 * h + 64, lo:512],
                                    V_sb[kb][:, (2 * p + h) * 64 : (2 * p + h + 1) * 64],
                                    pt[:, h, lo:512],
                                    start=(kb == 0),
                                    stop=last,
                                )
                            for h in range(2):
                                nc.tensor.matmul(
                                    zacc[64 * h : 64 * h + 64, lo:512],
                                    ones64,
                                    pt[:, h, lo:512],
                                    start=(kb == 0),
                                    stop=last,
                                )
                            if last:
                                zrec = stg.tile(
                                    [128, 512], F32, tag="zr", bufs=2,
                                    name=f"zr{p}{qc}",
                                )
                                nc.vector.reciprocal_approx_fast(zrec, zacc)
                                nc.vector.tensor_mul(
                                    AT[p][:, qc * 512 : (qc + 1) * 512],
                                    oacc,
                                    zrec,
                                )
                                if p == 1:
                                    for jt2 in range(4):
                                        for sub in range(2):
                                            proj_q.append((qc, jt2, sub))

                        # hard data deadlines: pop a filler unit and record
                        # what it produced; require() drains the queue until a
                        # needed product exists -- this guarantees producers
                        # are emitted strictly before their consumers
                        produced = made  # shared across phases

                        def pop_unit():
                            u = filler_q.pop(0)
                            u()
                            if u.produces is not None:
                                produced.add(u.produces)

                        def require(res):
                            while res not in produced:
                                assert filler_q, f"missing producer for {res}"
                                pop_unit()

                        for i in range(len(steps) + DEPTH):
                            if i < len(steps):
                                qc, kb = steps[i]
                                require(("qk", p, qc))
                                require(("qk", 2 + p, kb // 4))
                                emit_S(qc, kb)
                                # ~0.45us filler units hide in each step's
                                # tensor slack; short (diagonal) steps have
                                # room for two
                                if qc == 0 or 128 * (kb - 4 * qc) >= 128:
                                    budget = 2
                                else:
                                    budget = 1
                                for _ in range(budget):
                                    if filler_q:
                                        pop_unit()
                                    elif proj_q:
                                        emit_proj(*proj_q.pop(0))
                            else:
                                # flush region: drain remaining work
                                for _ in range(2):
                                    if filler_q:
                                        pop_unit()
                                    elif proj_q:
                                        emit_proj(*proj_q.pop(0), late=True)
                            j = i - DEPTH
                            if j >= 0:
                                require(("v", steps[j][1]))
                                emit_PV(*steps[j])
                        while filler_q:
                            pop_unit()
                        while proj_q:
                            emit_proj(*proj_q.pop(0), late=True)

                    # filler unit lists, ordered by data deadline against the
                    # ascending-qc step schedule (qT quarter n by qc_n start;
                    # kT quarter n by qc_n's kb=4n step; V[tt] by its first PV)
                    fillers0 = []
                    fillers0 += vg_units(0) + vg_units(1)
                    fillers0 += qkq_units(2, 1)
                    fillers0 += qkq_units(0, 2) + vg_units(2)
                    fillers0 += vg_units(3)
                    fillers0 += qkq_units(2, 2)
                    fillers0 += qkq_units(0, 3)
                    fillers0 += vg_units(4) + vg_units(5)
                    fillers0 += qkq_units(2, 3)
                    fillers0 += vg_units(6) + vg_units(7)
                    fillers0 += qkq_units(3, 0) + qkq_units(1, 0)
                    fillers1 = []
                    for mt, q in [(1, 1), (3, 1), (1, 2), (3, 2), (1, 3), (3, 3)]:
                        fillers1 += qkq_units(mt, q)
                    pair_phase(0, fillers0)
                    pair_phase(1, fillers1)

    nc.finalize()
    return nc


_NC_CACHE = None


def _get_nc():
    global _NC_CACHE
    if _NC_CACHE is None:
        _NC_CACHE = build_nc()
    return _NC_CACHE


def make_in_maps(x, w_qkv, w_proj):
    x = np.asarray(x, dtype=np.float32)
    w_qkv = np.asarray(w_qkv, dtype=np.float32)
    w_proj = np.asarray(w_proj, dtype=np.float32)
    in_maps = []
    for c in range(N_CORES):
        b, g = divmod(c, 4)
        cs = 256 * g
        in_maps.append(
            {
                "xT": np.ascontiguousarray(x[b].T).astype(NPBF),
                "wqk": np.ascontiguousarray(
                    np.concatenate(
                        [w_qkv[:, cs : cs + 256], w_qkv[:, D + cs : D + cs + 256]],
                        axis=1,
                    )
                ).astype(NPBF),
                "wv": np.ascontiguousarray(
                    w_qkv[:, 2 * D + cs : 2 * D + cs + 256]
                ).astype(NPBF),
                "wp": np.ascontiguousarray(w_proj[cs : cs + 256, :]).astype(NPBF),
            }
        )
    return in_maps


def assemble(results):
    out = np.empty((B, T, D), dtype=np.float32)
    for b in range(B):
        acc = results[4 * b]["outT"].astype(np.float32)
        for g in range(1, 4):
            acc = acc + results[4 * b + g]["outT"].astype(np.float32)
        for qc in range(4):
            out[b, qc * 512 : (qc + 1) * 512, :] = acc[qc].T
    return out


def kernel(x, w_qkv, w_proj, trace=False):
    nc = _get_nc()
    in_maps = make_in_maps(x, w_qkv, w_proj)
    res = bass_utils.run_bass_kernel_spmd(
        nc, in_maps, core_ids=list(range(N_CORES)), trace=trace
    )
    out = assemble(res.results)
    if trace:
        kernel.last_exec_time_ns = res.exec_time_ns
        kernel.last_result = res
    return out
